# revision 4
# baseline (speedup 1.0000x reference)
"""Trainium2 Bass kernel for the fuzzy-rule Controller model.

Model (hardcoded; see harness reference):
  B = 1_000_000, H = 64, 8 membership nets (2 actions x 4 state features).
  x = s[:, [0,1,2,3,0,1,2,3]]
  h1 = relu(x[:,n,None] * w1[n] + b1[n])          [B, n, 64]
  h2 = relu(h1 @ w2[n] + b2[n])                   [B, n, 64]
  z  = h2 @ w3[n] + b3[n]                         [B, n]
  m  = sigmoid(z); strength = min(m, groups of 4) [B, 2]
  out = softmax(strength * 5)

Mapping to 8 NeuronCores: pure data parallel over batch. Each core gets
125_000 rows padded to 125_440 = 245 tiles x 512.

Per tile (T=512 batch columns), nets are processed in pairs (i, i+4),
which share state feature i, block-stacked on the PE's 128 partitions:
  L1: matmul lhsT=[4,128] one-hot-row w1 block, rhs=sT[4,T]   -> PSUM[128,T]
  relu(+b1)  PSUM->SBUF (ScalarE for 2 pairs, VectorE for 2)
  L2: matmul lhsT=[128,128] block-diag w2,      rhs=h1[128,T] -> PSUM[128,T]
  relu(+b2)
  L3: matmul lhsT=[128,8] (cols 2i,2i+1 = w3),  rhs=h2        -> PSUM[8,T]
      all four pairs accumulate into one PSUM[8,T] (start=i==0, stop=i==3)
  z+b3 -> SBUF (ScalarE copy w/ bias), PE-transpose 4x [8,128]->[128,8]
  min-fold on [128,32] (batch-major), d = s0-s1, p0=sig(5d), p1=sig(-5d)
  staged [128, 8/tile] and flushed to DRAM every 35 tiles (1120B/partition).

All matmuls run as float32r (1 cycle/row at N=512 vs 4 for fp32).
Host side: transpose s shard -> sT[4, Bc]; un-permute outP[128, 245*8].
"""

import sys

sys.path.insert(0, "/opt/trn_rl_repo")

from contextlib import ExitStack

import numpy as np

import concourse.bacc as bacc
import concourse.bass as bass
import concourse.mybir as mybir
import concourse.tile as tile

F32 = mybir.dt.float32
F32R = mybir.dt.float32r
AF = mybir.ActivationFunctionType
ALU = mybir.AluOpType

H = 64
N_CORES = 8
B_TOTAL = 1_000_000
B_SHARD = B_TOTAL // N_CORES  # 125_000
T = 512  # batch columns per tile (= one fp32 PSUM bank)


def _build_program(n_tiles: int, flush_tiles: int):
    """Build + compile the single-core program (SPMD: same NEFF on all cores)."""
    assert n_tiles % flush_tiles == 0
    n_groups = n_tiles // flush_tiles
    bc = n_tiles * T

    nc = bacc.Bacc("TRN2", debug=False, target_bir_lowering=False)

    sT_d = nc.dram_tensor("sT", [4, bc], F32R, kind="ExternalInput")
    w1s_d = nc.dram_tensor("w1s", [4, 512], F32R, kind="ExternalInput")
    w2s_d = nc.dram_tensor("w2s", [128, 512], F32R, kind="ExternalInput")
    w3s_d = nc.dram_tensor("w3s", [128, 32], F32R, kind="ExternalInput")
    b1s_d = nc.dram_tensor("b1s", [128, 4], F32, kind="ExternalInput")
    b2s_d = nc.dram_tensor("b2s", [128, 4], F32, kind="ExternalInput")
    b3p_d = nc.dram_tensor("b3p", [8, 1], F32, kind="ExternalInput")
    id8_d = nc.dram_tensor("id8", [8, 8], F32, kind="ExternalInput")
    outP_d = nc.dram_tensor("outP", [128, n_tiles * 8], F32, kind="ExternalOutput")

    with tile.TileContext(nc) as tc, ExitStack() as ctx:
        wp = ctx.enter_context(tc.tile_pool(name="w", bufs=1))
        inp = ctx.enter_context(tc.tile_pool(name="in", bufs=3))
        hp = ctx.enter_context(tc.tile_pool(name="h", bufs=3))
        zp = ctx.enter_context(tc.tile_pool(name="zs", bufs=2))
        tp_ = ctx.enter_context(tc.tile_pool(name="tail", bufs=2))
        sp = ctx.enter_context(tc.tile_pool(name="stg", bufs=2))
        pA = ctx.enter_context(tc.tile_pool(name="pA", bufs=2, space="PSUM"))
        pB = ctx.enter_context(tc.tile_pool(name="pB", bufs=2, space="PSUM"))
        pZ = ctx.enter_context(tc.tile_pool(name="pZ", bufs=2, space="PSUM"))
        pT = ctx.enter_context(tc.tile_pool(name="pT", bufs=2, space="PSUM"))

        w1t = wp.tile([4, 512], F32R)
        nc.sync.dma_start(w1t[:], w1s_d.ap()[:])
        w2t = wp.tile([128, 512], F32R)
        nc.sync.dma_start(w2t[:], w2s_d.ap()[:])
        w3t = wp.tile([128, 32], F32R)
        nc.sync.dma_start(w3t[:], w3s_d.ap()[:])
        b1t = wp.tile([128, 4], F32)
        nc.sync.dma_start(b1t[:], b1s_d.ap()[:])
        b2t = wp.tile([128, 4], F32)
        nc.sync.dma_start(b2t[:], b2s_d.ap()[:])
        b3t = wp.tile([8, 1], F32)
        nc.sync.dma_start(b3t[:], b3p_d.ap()[:])
        id8t = wp.tile([8, 8], F32)
        nc.sync.dma_start(id8t[:], id8_d.ap()[:])

        for grp in range(n_groups):
            stg = sp.tile([128, flush_tiles * 8], F32)
            stgv = stg[:].rearrange("p (t j a) -> p t j a", t=flush_tiles, j=4, a=2)
            for tl in range(flush_tiles):
                t = grp * flush_tiles + tl
                st = inp.tile([4, T], F32R, tag="st")
                nc.sync.dma_start(st[:], sT_d.ap()[:, t * T : (t + 1) * T])

                zps = pZ.tile([8, T], F32)
                for i in range(4):
                    a = pA.tile([128, T], F32)
                    nc.tensor.matmul(
                        a[:],
                        w1t[:, 128 * i : 128 * (i + 1)],
                        st[:],
                        start=True,
                        stop=True,
                    )
                    h1 = hp.tile([128, T], F32R, tag="h1")
                    if i % 2 == 0:
                        nc.scalar.activation(
                            h1[:], a[:], AF.Relu, bias=b1t[:, i : i + 1]
                        )
                    else:
                        nc.vector.tensor_scalar(
                            h1[:], a[:], b1t[:, i : i + 1], 0.0, ALU.add, ALU.max
                        )
                    b = pB.tile([128, T], F32)
                    nc.tensor.matmul(
                        b[:],
                        w2t[:, 128 * i : 128 * (i + 1)],
                        h1[:],
                        start=True,
                        stop=True,
                    )
                    h2 = hp.tile([128, T], F32R, tag="h2")
                    if i % 2 == 0:
                        nc.scalar.activation(
                            h2[:], b[:], AF.Relu, bias=b2t[:, i : i + 1]
                        )
                    else:
                        nc.vector.tensor_scalar(
                            h2[:], b[:], b2t[:, i : i + 1], 0.0, ALU.add, ALU.max
                        )
                    nc.tensor.matmul(
                        zps[:],
                        w3t[:, 8 * i : 8 * (i + 1)],
                        h2[:],
                        start=(i == 0),
                        stop=(i == 3),
                    )

                # z + b3 -> SBUF, then transpose to batch-major [128, 32]
                zs = zp.tile([8, T], F32)
                nc.scalar.activation(zs[:], zps[:], AF.Identity, bias=b3t[:])
                tpm = pT.tile([128, 32], F32)
                for j in range(4):
                    nc.tensor.transpose(
                        tpm[:, 8 * j : 8 * (j + 1)],
                        zs[:, 128 * j : 128 * (j + 1)],
                        id8t[:],
                    )
                V = tp_.tile([128, 32], F32, tag="V")
                nc.vector.tensor_copy(V[:], tpm[:])
                # V[p, j, i, a]: batch j*128+p, pair i, action a
                V4 = V[:].rearrange("p (j i a) -> p j i a", j=4, i=4, a=2)
                M1 = tp_.tile([128, 16], F32, tag="M1")
                M14 = M1[:].rearrange("p (j i a) -> p j i a", j=4, i=2, a=2)
                nc.vector.tensor_tensor(
                    M14, V4[:, :, 0:2, :], V4[:, :, 2:4, :], ALU.min
                )
                S = tp_.tile([128, 8], F32, tag="S")
                S4 = S[:].rearrange("p (j a) -> p j a", j=4, a=2)
                nc.vector.tensor_tensor(
                    S4, M14[:, :, 0:1, :], M14[:, :, 1:2, :], ALU.min
                )
                SS = tp_.tile([128, 8], F32, tag="SS")
                SS4 = SS[:].rearrange("p (j a) -> p j a", j=4, a=2)
                nc.scalar.activation(SS[:], S[:], AF.Sigmoid)
                D = tp_.tile([128, 4], F32, tag="D")
                nc.vector.tensor_tensor(
                    D[:], SS4[:, :, 0:1], SS4[:, :, 1:2], ALU.subtract
                )
                # softmax over 2 actions: p0 = sigmoid(5d), p1 = sigmoid(-5d)
                nc.scalar.activation(
                    stgv[:, tl, :, 0:1], D[:], AF.Sigmoid, scale=5.0
                )
                nc.scalar.activation(
                    stgv[:, tl, :, 1:2], D[:], AF.Sigmoid, scale=-5.0
                )
            nc.sync.dma_start(
                outP_d.ap()[:, grp * flush_tiles * 8 : (grp + 1) * flush_tiles * 8],
                stg[:],
            )

    nc.compile()
    return nc


def _pack_weights(w1, b1, w2, b2, w3, b3):
    w1 = np.asarray(w1, np.float32)
    b1 = np.asarray(b1, np.float32)
    w2 = np.asarray(w2, np.float32)
    b2 = np.asarray(b2, np.float32)
    w3 = np.asarray(w3, np.float32)
    b3 = np.asarray(b3, np.float32)
    w1s = np.zeros((4, 512), np.float32)
    w2s = np.zeros((128, 512), np.float32)
    w3s = np.zeros((128, 32), np.float32)
    b1s = np.zeros((128, 4), np.float32)
    b2s = np.zeros((128, 4), np.float32)
    b3p = np.zeros((8, 1), np.float32)
    for i in range(4):
        w1s[i, 128 * i : 128 * i + 64] = w1[i]
        w1s[i, 128 * i + 64 : 128 * (i + 1)] = w1[i + 4]
        w2s[0:64, 128 * i : 128 * i + 64] = w2[i]
        w2s[64:128, 128 * i + 64 : 128 * (i + 1)] = w2[i + 4]
        w3s[0:64, 8 * i + 2 * i] = w3[i]
        w3s[64:128, 8 * i + 2 * i + 1] = w3[i + 4]
        b1s[0:64, i] = b1[i]
        b1s[64:128, i] = b1[i + 4]
        b2s[0:64, i] = b2[i]
        b2s[64:128, i] = b2[i + 4]
        b3p[2 * i, 0] = b3[i]
        b3p[2 * i + 1, 0] = b3[i + 4]
    id8 = np.eye(8, dtype=np.float32)
    return dict(w1s=w1s, w2s=w2s, w3s=w3s, b1s=b1s, b2s=b2s, b3p=b3p, id8=id8)


def _make_in_maps(s, weights, n_tiles):
    s = np.asarray(s, np.float32)
    bc = n_tiles * T
    in_maps = []
    for c in range(N_CORES):
        shard = s[c * B_SHARD : (c + 1) * B_SHARD]
        sT = np.zeros((4, bc), np.float32)
        sT[:, : shard.shape[0]] = shard.T
        in_maps.append(dict(weights, sT=np.ascontiguousarray(sT)))
    return in_maps


def _unpack_out(results, n_tiles):
    bc = n_tiles * T
    out = np.empty((B_TOTAL, 2), np.float32)
    for c in range(N_CORES):
        outP = results[c]["outP"]  # [128, n_tiles*8]
        full = (
            outP.reshape(128, n_tiles, 4, 2)
            .transpose(1, 2, 0, 3)
            .reshape(bc, 2)
        )
        out[c * B_SHARD : (c + 1) * B_SHARD] = full[:B_SHARD]
    return out


_NC_CACHE = {}


def _get_program(n_tiles=245, flush_tiles=35):
    key = (n_tiles, flush_tiles)
    if key not in _NC_CACHE:
        _NC_CACHE[key] = _build_program(n_tiles, flush_tiles)
    return _NC_CACHE[key]


def run(s, w1, b1, w2, b2, w3, b3, trace=False, n_tiles=245, flush_tiles=35):
    from concourse.bass_utils import run_bass_kernel_spmd

    nc = _get_program(n_tiles, flush_tiles)
    weights = _pack_weights(w1, b1, w2, b2, w3, b3)
    in_maps = _make_in_maps(s, weights, n_tiles)
    res = run_bass_kernel_spmd(
        nc, in_maps, core_ids=list(range(N_CORES)), trace=trace
    )
    return _unpack_out(res.results, n_tiles), res


def kernel(s, w1, b1, w2, b2, w3, b3):
    out, _ = run(s, w1, b1, w2, b2, w3, b3)
    return out
